# revision 6
# baseline (speedup 1.0000x reference)
"""Bass/Tile TRN2 kernel for a 3x3 locally-connected (unshared-weight) layer.

Computation (per batch row b, grid unit h, hw = 256*256):
    y[b,h] = sigmoid( sum_o x[b, nbr_idx[o,h]] * (valid[o,h] ? weights[o,h] : 0) )
    y[b,h] = sigmoid(0) = 0.5 where ~fault_mask[h] (mask applied pre-sigmoid)

Strategy: the gather is a fixed 3x3 stencil (verified on host at call time).
The grid (256x256) is tiled into 8x16 output patches (128 outputs = full PE
width).  A patch's 9-point stencil inputs form its 10x18 hull (180 grid
cells); with x transposed to (cell, batch), each patch is TWO matmuls:
    psum[128 out, 256 batch]  = lhsT_A[116 hull-rows, 128].T @ xh_A[116, 256]
    psum                     += lhsT_B[ 64 hull-rows, 128].T @ xh_B[ 64, 256]
where the lhsT blocks hold the (mostly zero) scattered effective weights.
The kernel is HBM-DMA-bound, so bytes-on-the-wire are the currency:
  - x hulls ship as fp8 e3m4 (x is pre-scaled by 2 on host; the ACT's
    scale=0.5 descales).  4 mantissa bits keep rel_err ~1.4e-2 < 2e-2.
  - weight blocks stay bf16 (fp8 for both operands breaks the error gate).
  - the output ships as int8: ScalarE sigmoid -> bf16, then the (otherwise
    idle) Vector engine quantizes (sigmoid*480 - 240) -> int8; the host
    dequantizes q/480 + 0.5.  Halves the 4 MiB output stream.
  - the 180-row hull splits 116+64 (not 128+52): B-halves of two groups
    pack one 128-row tile exactly, so no pad rows ship.
Traffic: ~7.4 MiB/core vs ~13.9 for the all-bf16 formulation.

Sharding: gy is split 8 ways (32 grid rows = 4 patch-rows of 16 patches per
core); batch (256) rides along the matmul free dimension.  Every core runs
an identical program; grid-boundary effects are encoded in host-built
zero-padded hulls / zero weight blocks.  All inputs are SBUF-resident and
DMA'd up-front in consumption order, balanced across FOUR dynamic DMA rings
(sync/scalar/vector HWDGE + gpsimd SWDGE).  Scheduling rules baked in:
matmuls run in same-shape runs so LDWEIGHTS pipelines; the Scalar/ACT
engine issues few input DMAs and no stores (a dma_start blocked on ring
capacity stalls every later ACTIVATE in its FIFO); a tiny early ACTIVATE
hoists the sigmoid ACT_TABLE_LOAD (~1.5 us) off the serial sigmoid chain;
one 4-bank ACT per group keeps that chain short; dummy matmuls after the
first three groups hold the PE's HAM clock-gate at 8/8 through the
input-starved ramp.
"""

import numpy as np
import ml_dtypes

BATCH = 256
W = 256               # grid width/height
HW = W * W
N_CONN = 9
PA, PB = 8, 16        # patch shape (gy x gx) -> M = 128 outputs
HA, HB = PA + 2, PB + 2   # hull shape 10 x 18 -> K = 180, split 116 + 64
KSPLIT = 116
K2 = HA * HB - KSPLIT     # 64: two B-halves pack one 128-row tile exactly
NPAIR_G = 4           # group pairs per core
NCORES = 8
NPY, NPX = W // PA, W // PB      # 32 x 16 patch grid
PRPC = NPY // NCORES             # 4 patch-rows per core
NGRP = PRPC * 2                  # 8 half-row DMA groups (8 patches each)
GP = NPX // 2                    # patches per group
NPATCH = PRPC * NPX              # 64 patches per core

XSCALE = 2.0          # host pre-scale before e3m4 cast; ACT descales
QSCALE = 480.0        # int8 output quant: q = sigmoid*QSCALE - QSCALE/2

_BF16 = ml_dtypes.bfloat16
_F8E3 = ml_dtypes.float8_e3m4


def _build_patch_weights(weights, nbr_idx, valid):
    """Scatter effective weights into per-patch lhsT blocks.

    Returns W4 float32 (NPY*NPX, HA*HB, 128): for patch P, W4[P, k, m] is the
    weight of the connection feeding output m (= oy*16+ox) from hull cell k
    (= hy*18+hx, hull origin one cell up-left of the patch).  Raises
    ValueError if some valid (o,h) connection is not coverable.
    """
    h = np.arange(HW, dtype=np.int64)
    gy, gx = h // W, h % W
    PY, PX = gy // PA, gx // PB
    P = PY * NPX + PX
    m = (gy % PA) * PB + (gx % PB)

    g = nbr_idx.astype(np.int64)
    vm = valid.astype(bool)
    w_eff = np.where(vm, weights.astype(np.float32), 0.0)

    hy = g // W - (PA * PY - 1)
    hx = g % W - (PB * PX - 1)
    inh = (hy >= 0) & (hy < HA) & (hx >= 0) & (hx < HB)
    if not np.all(inh | ~vm):
        raise ValueError(
            "nbr_idx is not coverable by the patch-stencil kernel "
            f"({np.count_nonzero(vm & ~inh)} uncovered connections)"
        )
    k = hy * HB + hx
    mask = vm & inh
    Pb = np.broadcast_to(P, g.shape)
    mb = np.broadcast_to(m, g.shape)
    W4 = np.zeros((NPY * NPX, HA * HB, 128), dtype=np.float32)
    np.add.at(W4, (Pb[mask], k[mask], mb[mask]), w_eff[mask])
    return W4


def _build_program():
    import concourse.bacc as bacc
    import concourse.mybir as mybir
    from concourse import tile
    from concourse._compat import axon_active

    nc = bacc.Bacc(
        "TRN2",
        target_bir_lowering=False,
        debug=not axon_active(),
        num_devices=NCORES,
    )
    f32 = mybir.dt.float32
    bf16 = mybir.dt.bfloat16
    f8e3 = mybir.dt.float8e3
    i8 = mybir.dt.int8

    xh1_d = nc.dram_tensor("xh1", [NGRP, KSPLIT, GP * 256], f8e3, kind="ExternalInput")
    xh2_d = nc.dram_tensor("xh2", [NPAIR_G, 128, GP * 256], f8e3, kind="ExternalInput")
    wt1_d = nc.dram_tensor("wt1", [NGRP, KSPLIT, GP * 128], bf16, kind="ExternalInput")
    wt2_d = nc.dram_tensor("wt2", [NPAIR_G, 128, GP * 128], bf16, kind="ExternalInput")
    yq_d = nc.dram_tensor("yq", [128, NPATCH * 256], i8, kind="ExternalOutput")

    with tile.TileContext(nc) as tc:
        with (
            tc.tile_pool(name="xh", bufs=1) as xh_pool,
            tc.tile_pool(name="wt", bufs=1) as wt_pool,
            tc.tile_pool(name="const", bufs=1) as const_pool,
            tc.tile_pool(name="out", bufs=3) as out_pool,
            tc.tile_pool(name="oq", bufs=4) as oq_pool,
            tc.tile_pool(name="psum", bufs=2, space="PSUM") as psum_pool,
        ):
            # All inputs SBUF-resident; every DMA issued up-front in
            # consumption order, balanced across the four dynamic rings.
            xh1_sb, wt1_sb, xh2_sb, wt2_sb = [], [], [], []
            for g in range(NGRP):
                xh1_sb.append(xh_pool.tile([KSPLIT, GP * 256], f8e3, tag=f"xh1_{g}", name=f"xh1_{g}"))
                wt1_sb.append(wt_pool.tile([KSPLIT, GP * 128], bf16, tag=f"wt1_{g}", name=f"wt1_{g}"))
            for p in range(NPAIR_G):
                xh2_sb.append(xh_pool.tile([128, GP * 256], f8e3, tag=f"xh2_{p}", name=f"xh2_{p}"))
                wt2_sb.append(wt_pool.tile([128, GP * 128], bf16, tag=f"wt2_{p}", name=f"wt2_{p}"))
            srcs = {"xh1": (xh1_d, xh1_sb), "xh2": (xh2_d, xh2_sb),
                    "wt1": (wt1_d, wt1_sb), "wt2": (wt2_d, wt2_sb)}
            # sync ring: first-consumed tensors (leads the ramp)
            for t, idx in (("xh1", 0), ("wt1", 0), ("wt2", 0), ("xh1", 2),
                           ("wt1", 3), ("xh1", 4), ("wt1", 5), ("xh1", 7),
                           ("wt1", 7)):
                src_d, dst = srcs[t]
                nc.sync.dma_start(out=dst[idx][:, :], in_=src_d[idx])
            # scalar ring: few DMAs, all preceding every ACTIVATE; a tiny
            # ACT between them pulls the sigmoid table load forward
            for t, idx in (("xh2", 0), ("xh1", 1), ("xh2", 1), ("xh1", 3)):
                src_d, dst = srcs[t]
                nc.scalar.dma_start(out=dst[idx][:, :], in_=src_d[idx])
            # gpsimd SWDGE: ~3.5 us to first byte; later-consumed tensors
            for t, idx in (("wt1", 1), ("wt1", 2), ("wt2", 1), ("wt1", 4),
                           ("wt2", 2), ("xh1", 5), ("xh1", 6), ("wt2", 3),
                           ("wt1", 6)):
                src_d, dst = srcs[t]
                nc.gpsimd.dma_start(out=dst[idx][:, :], in_=src_d[idx])

            # PE pre-warm: dummy matmuls on zeroed SBUF while the first input
            # DMAs are in flight, so the HAM clock-gate opens (1.2 -> 2.4 GHz)
            # before the real matmul stream begins.
            warm_sb = const_pool.tile([128, 640], bf16, tag="warm")
            nc.vector.memset(warm_sb[:, :], 0.0)
            act_probe = const_pool.tile([1, 2], f32, tag="act_probe")
            warm_ps = psum_pool.tile([128, 2048], f32, tag="pA", name="warm_ps")
            for _ in range(10):
                nc.tensor.matmul(
                    warm_ps[:, 0:512],
                    warm_sb[:, 0:128],
                    warm_sb[:, 128:640],
                    start=True,
                    stop=True,
                )
            # Hoist the sigmoid ACT_TABLE_LOAD off the serial sigmoid chain:
            # the assembler emits the table load right before this probe,
            # which runs ~8 us in, instead of right before group 0's ACT.
            nc.scalar.activation(
                act_probe[:, :],
                warm_sb[0:1, 0:2],
                mybir.ActivationFunctionType.Sigmoid,
                bias=0.0,
                scale=0.5,
            )
            # scalar ring, remainder (still before every real ACTIVATE)
            for t, idx in (("xh2", 2), ("xh2", 3)):
                src_d, dst = srcs[t]
                nc.scalar.dma_start(out=dst[idx][:, :], in_=src_d[idx])

            # Per group: eight K=116 A-matmuls back-to-back, then eight K=64
            # B-matmuls.  Identical LDWEIGHTS shapes keep the PE's
            # background-weight-buffer pull-ahead alive (~131 ns/MM vs ~390).
            store_eng = [nc.sync, nc.gpsimd, nc.sync, nc.gpsimd,
                         nc.sync, nc.gpsimd, nc.sync, None]
            final_store_eng = [nc.sync, nc.gpsimd]
            for g in range(NGRP):
                b0 = (g % 2) * K2  # B-half base partition within the pair tile
                ps = psum_pool.tile([128, 2048], f32, tag="pA", name=f"ps_{g}")
                for px in range(GP):
                    co = px * 256
                    # start=True on each 512-wide bank's first MM clears that
                    # bank's has_written bits; later MMs (start=False)
                    # overwrite fresh cells and accumulate onto written ones.
                    nc.tensor.matmul(
                        ps[:, co : co + 256],
                        wt1_sb[g][:, px * 128 : (px + 1) * 128],
                        xh1_sb[g][:, px * 256 : (px + 1) * 256],
                        start=(px % 2 == 0),
                        stop=False,
                        skip_group_check=True,
                    )
                ot = out_pool.tile([128, 2048], bf16)
                oq = oq_pool.tile([128, 2048], i8)
                for px in range(GP):
                    co = px * 256
                    nc.tensor.matmul(
                        ps[:, co : co + 256],
                        wt2_sb[g // 2][b0 : b0 + K2, px * 128 : (px + 1) * 128],
                        xh2_sb[g // 2][b0 : b0 + K2, px * 256 : (px + 1) * 256],
                        start=False,
                        stop=(px % 2 == 1),
                        skip_group_check=True,
                    )
                # Post-processing chain per group: one 4-bank sigmoid on the
                # Scalar engine (scale=0.5 descales the x*2 host pre-scale),
                # then the Vector engine quantizes to int8 for the store.
                if g == NGRP - 1:
                    # split the final group so the tail is half-sized stages
                    for h in range(2):
                        sl = slice(h * 1024, (h + 1) * 1024)
                        nc.scalar.activation(
                            ot[:, sl], ps[:, sl],
                            mybir.ActivationFunctionType.Sigmoid,
                            bias=0.0, scale=1.0 / XSCALE,
                        )
                        nc.vector.tensor_scalar(
                            oq[:, sl], ot[:, sl],
                            QSCALE, -QSCALE / 2,
                            mybir.AluOpType.mult, mybir.AluOpType.add,
                        )
                        final_store_eng[h].dma_start(
                            out=yq_d[:, g * 2048 + h * 1024 : g * 2048 + (h + 1) * 1024],
                            in_=oq[:, sl],
                        )
                else:
                    nc.scalar.activation(
                        ot[:, 0:2048], ps[:, 0:2048],
                        mybir.ActivationFunctionType.Sigmoid,
                        bias=0.0, scale=1.0 / XSCALE,
                    )
                    nc.vector.tensor_scalar(
                        oq[:, 0:2048], ot[:, 0:2048],
                        QSCALE, -QSCALE / 2,
                        mybir.AluOpType.mult, mybir.AluOpType.add,
                    )
                    store_eng[g].dma_start(
                        out=yq_d[:, g * 2048 : (g + 1) * 2048],
                        in_=oq[:, :],
                    )
                if g < 3:
                    # HAM-warmkeeping filler: dummy matmuls occupy the PE
                    # through the early input-starved gaps so the clock gate
                    # stays at 8/8.  Their garbage lands in a PSUM slot that
                    # a later group's start=True matmul clears anyway.
                    for _ in range(8):
                        nc.tensor.matmul(
                            warm_ps[:, 0:512],
                            warm_sb[:, 0:128],
                            warm_sb[:, 128:640],
                            start=True,
                            stop=True,
                        )
    nc.compile()
    return nc


TRACE = False          # set by test harness to capture an NTFF profile
LAST_RESULTS = None    # BassKernelResults of the most recent run
_NC_CACHE = None       # compiled program, reused across calls


def kernel(x, weights, nbr_idx, valid, fault_mask):
    global LAST_RESULTS
    from concourse.bass_utils import run_bass_kernel_spmd

    x = np.asarray(x)
    out_dtype = x.dtype

    W4 = _build_patch_weights(
        np.asarray(weights), np.asarray(nbr_idx), np.asarray(valid)
    ).astype(_BF16)

    # x -> zero-padded (258, 258, B) grid, fp8 e3m4 scaled by XSCALE
    xtp = np.zeros((W + 2, W + 2, BATCH), dtype=_F8E3)
    xs = np.clip(np.ascontiguousarray(x.T).astype(np.float32) * XSCALE, -15.5, 15.5)
    xtp[1 : W + 1, 1 : W + 1] = xs.astype(_F8E3).reshape(W, W, BATCH)
    # all patch hulls: (NPY, NPX, HA*HB, B)
    sl = np.lib.stride_tricks.sliding_window_view(xtp, (HA, HB), axis=(0, 1))
    hulls = (
        sl[::PA, ::PB]                      # (NPY, NPX, B, HA, HB)
        .transpose(0, 1, 3, 4, 2)
        .reshape(NPY, NPX, HA * HB, BATCH)
    )

    W4 = W4.reshape(NPY, NPX, HA * HB, 128)
    in_maps = []
    for c in range(NCORES):
        hc = hulls[c * PRPC : (c + 1) * PRPC]   # (PRPC, NPX, 180, B)
        wc = W4[c * PRPC : (c + 1) * PRPC]      # (PRPC, NPX, 180, 128)
        # half-row groups of GP=8 patches: [NGRP, 180, GP, .]
        hg = hc.reshape(NGRP, GP, HA * HB, BATCH).transpose(0, 2, 1, 3)
        wg = wc.reshape(NGRP, GP, HA * HB, 128).transpose(0, 2, 1, 3)
        # B-halves: two groups' 64 rows pack one 128-row tile exactly
        hb = hg[:, KSPLIT:].reshape(NPAIR_G, 2 * K2, GP, BATCH)
        wb = wg[:, KSPLIT:].reshape(NPAIR_G, 2 * K2, GP, 128)
        in_maps.append(
            {
                "xh1": np.ascontiguousarray(hg[:, :KSPLIT]).reshape(
                    NGRP, KSPLIT, GP * 256
                ),
                "xh2": np.ascontiguousarray(hb).reshape(NPAIR_G, 128, GP * 256),
                "wt1": np.ascontiguousarray(wg[:, :KSPLIT]).reshape(
                    NGRP, KSPLIT, GP * 128
                ),
                "wt2": np.ascontiguousarray(wb).reshape(NPAIR_G, 128, GP * 128),
            }
        )

    global _NC_CACHE
    if _NC_CACHE is None:
        _NC_CACHE = _build_program()
    nc = _NC_CACHE
    res = run_bass_kernel_spmd(
        nc, in_maps, core_ids=list(range(NCORES)), trace=TRACE
    )
    LAST_RESULTS = res

    # unshard: per-core yq is [m=oy*16+ox, NPATCH*256] int8 with patches in
    # (patch-row-major, quad) order -> dequant -> (B, HW)
    parts = []
    for c, r in enumerate(res.results):
        yq = np.asarray(r["yq"]).reshape(PA, PB, PRPC, NPX, BATCH)
        # [oy, ox, pyl, px, b] -> [b, pyl, oy, px, ox]
        parts.append(
            yq.transpose(4, 2, 0, 3, 1).reshape(BATCH, PRPC * PA, W)
        )
    yq_full = np.concatenate(parts, axis=1).reshape(BATCH, HW)
    y = (yq_full.astype(np.float32) / QSCALE + np.float32(0.5)).astype(
        out_dtype, copy=False
    )
    # faulted units: reference computes sigmoid(where(fault, y, 0)) -> 0.5
    fault = np.asarray(fault_mask).astype(bool)
    y[:, ~fault] = np.float32(0.5)
    return y


# revision 10
# speedup vs baseline: 1.2604x; 1.2604x over previous
"""Bass/Tile TRN2 kernel for a 3x3 locally-connected (unshared-weight) layer.

Computation (per batch row b, grid unit h, hw = 256*256):
    y[b,h] = sigmoid( sum_o x[b, nbr_idx[o,h]] * (valid[o,h] ? weights[o,h] : 0) )
    y[b,h] = sigmoid(0) = 0.5 where ~fault_mask[h] (mask applied pre-sigmoid)

Strategy: the gather is a fixed 3x3 stencil (verified on host at call time).
The grid (256x256) is tiled into 8x16 output patches (128 outputs = full PE
width).  A patch's 9-point stencil inputs form its 10x18 hull (180 grid
cells); with x transposed to (cell, batch), each patch is TWO matmuls:
    psum[128 out, 256 batch]  = lhsT_A[128 hull-rows, 128].T @ xh_A[128, 256]
    psum                     += lhsT_B[ 52 hull-rows, 128].T @ xh_B[ 52, 256]
where the lhsT blocks hold the (mostly zero) scattered effective weights.
The kernel is HBM-DMA-bound, so bytes-on-the-wire are the currency:
  - x hulls ship as fp8 e3m4 (x is pre-scaled by 2 on host; the ACT's
    scale=0.5 descales).  4 mantissa bits keep rel_err ~1.4e-2 < 2e-2.
  - weight blocks stay bf16 (fp8 for both operands breaks the error gate).
  - the output ships as int8: ScalarE sigmoid -> bf16, then the (otherwise
    idle) Vector engine quantizes (sigmoid*480 - 240) -> int8; the host
    dequantizes q/480 + 0.5.  Halves the 4 MiB output stream.
  - every DMA tile keeps 128 partitions: the DGE splits a transfer into
    per-partition-group descriptors and non-128 partition counts skew the
    descriptor->channel spread, hot-spotting a few of the 16 HW channels.
Traffic: ~8.0 MiB/core vs ~13.9 for the all-bf16 formulation.

Sharding: gy is split 8 ways (32 grid rows = 4 patch-rows of 16 patches per
core); batch (256) rides along the matmul free dimension.  Every core runs
an identical program; grid-boundary effects are encoded in host-built
zero-padded hulls / zero weight blocks.  All inputs are SBUF-resident and
DMA'd up-front in consumption order, balanced across FOUR dynamic DMA rings
(sync/scalar/vector HWDGE + gpsimd SWDGE).  Scheduling rules baked in:
matmuls run in same-shape runs so LDWEIGHTS pipelines; the Scalar/ACT
engine issues few input DMAs and no stores (a dma_start blocked on ring
capacity stalls every later ACTIVATE in its FIFO); a tiny early ACTIVATE
hoists the sigmoid ACT_TABLE_LOAD (~1.5 us) off the serial sigmoid chain;
one 4-bank ACT per group keeps that chain short; dummy matmuls after the
first three groups hold the PE's HAM clock-gate at 8/8 through the
input-starved ramp.
"""

import numpy as np
import ml_dtypes

BATCH = 256
W = 256               # grid width/height
HW = W * W
N_CONN = 9
PA, PB = 8, 16        # patch shape (gy x gx) -> M = 128 outputs
HA, HB = PA + 2, PB + 2   # hull shape 10 x 18 -> K = 180, split 128 + 52
KSPLIT = 128
K2 = HA * HB - KSPLIT     # 52
K2P = 64              # B-half padded stride: two groups pack one 128-row tile
NPAIR_G = 4           # group pairs per core
NCORES = 8
NPY, NPX = W // PA, W // PB      # 32 x 16 patch grid
PRPC = NPY // NCORES             # 4 patch-rows per core
NGRP = PRPC * 2                  # 8 half-row DMA groups (8 patches each)
GP = NPX // 2                    # patches per group
NPATCH = PRPC * NPX              # 64 patches per core

XSCALE = 2.0          # host pre-scale before e3m4 cast; ACT descales
QSCALE = 480.0        # int8 output quant: q = sigmoid*QSCALE - QSCALE/2

_BF16 = ml_dtypes.bfloat16
_F8E3 = ml_dtypes.float8_e3m4


def _build_patch_weights(weights, nbr_idx, valid):
    """Scatter effective weights into per-patch lhsT blocks.

    Returns W4 float32 (NPY*NPX, HA*HB, 128): for patch P, W4[P, k, m] is the
    weight of the connection feeding output m (= oy*16+ox) from hull cell k
    (= hy*18+hx, hull origin one cell up-left of the patch).  Raises
    ValueError if some valid (o,h) connection is not coverable.
    """
    h = np.arange(HW, dtype=np.int64)
    gy, gx = h // W, h % W
    PY, PX = gy // PA, gx // PB
    P = PY * NPX + PX
    m = (gy % PA) * PB + (gx % PB)

    g = nbr_idx.astype(np.int64)
    vm = valid.astype(bool)
    w_eff = np.where(vm, weights.astype(np.float32), 0.0)

    hy = g // W - (PA * PY - 1)
    hx = g % W - (PB * PX - 1)
    inh = (hy >= 0) & (hy < HA) & (hx >= 0) & (hx < HB)
    if not np.all(inh | ~vm):
        raise ValueError(
            "nbr_idx is not coverable by the patch-stencil kernel "
            f"({np.count_nonzero(vm & ~inh)} uncovered connections)"
        )
    k = hy * HB + hx
    mask = vm & inh
    Pb = np.broadcast_to(P, g.shape)
    mb = np.broadcast_to(m, g.shape)
    W4 = np.zeros((NPY * NPX, HA * HB, 128), dtype=np.float32)
    np.add.at(W4, (Pb[mask], k[mask], mb[mask]), w_eff[mask])
    return W4


def _build_program():
    import concourse.bacc as bacc
    import concourse.mybir as mybir
    from concourse import tile
    from concourse._compat import axon_active

    nc = bacc.Bacc(
        "TRN2",
        target_bir_lowering=False,
        debug=not axon_active(),
        num_devices=NCORES,
    )
    f32 = mybir.dt.float32
    bf16 = mybir.dt.bfloat16
    f8e3 = mybir.dt.float8e3
    i8 = mybir.dt.int8

    xh1_d = nc.dram_tensor("xh1", [NGRP, KSPLIT, GP * 256], f8e3, kind="ExternalInput")
    xh2_d = nc.dram_tensor("xh2", [NPAIR_G, 128, GP * 256], f8e3, kind="ExternalInput")
    wt1_d = nc.dram_tensor("wt1", [NGRP, KSPLIT, GP * 128], bf16, kind="ExternalInput")
    wt2_d = nc.dram_tensor("wt2", [NPAIR_G, 128, GP * 128], bf16, kind="ExternalInput")
    yq_d = nc.dram_tensor("yq", [128, NPATCH * 256], i8, kind="ExternalOutput")

    with tile.TileContext(nc) as tc:
        with (
            tc.tile_pool(name="xh", bufs=1) as xh_pool,
            tc.tile_pool(name="wt", bufs=1) as wt_pool,
            tc.tile_pool(name="const", bufs=1) as const_pool,
            tc.tile_pool(name="out", bufs=3) as out_pool,
            tc.tile_pool(name="oq", bufs=4) as oq_pool,
            tc.tile_pool(name="psum", bufs=2, space="PSUM") as psum_pool,
        ):
            # All inputs SBUF-resident; every DMA issued up-front in
            # consumption order, balanced across the four dynamic rings.
            xh1_sb, wt1_sb, xh2_sb, wt2_sb = [], [], [], []
            for g in range(NGRP):
                xh1_sb.append(xh_pool.tile([KSPLIT, GP * 256], f8e3, tag=f"xh1_{g}", name=f"xh1_{g}"))
                wt1_sb.append(wt_pool.tile([KSPLIT, GP * 128], bf16, tag=f"wt1_{g}", name=f"wt1_{g}"))
            for p in range(NPAIR_G):
                xh2_sb.append(xh_pool.tile([128, GP * 256], f8e3, tag=f"xh2_{p}", name=f"xh2_{p}"))
                wt2_sb.append(wt_pool.tile([128, GP * 128], bf16, tag=f"wt2_{p}", name=f"wt2_{p}"))
            srcs = {"xh1": (xh1_d, xh1_sb), "xh2": (xh2_d, xh2_sb),
                    "wt1": (wt1_d, wt1_sb), "wt2": (wt2_d, wt2_sb)}
            # sync ring: first-consumed tensors (leads the ramp)
            for t, idx in (("xh1", 0), ("wt1", 0), ("wt2", 0), ("xh1", 2),
                           ("wt1", 3), ("xh1", 4), ("wt1", 5), ("xh1", 7)):
                src_d, dst = srcs[t]
                nc.sync.dma_start(out=dst[idx][:, :], in_=src_d[idx])
            # scalar ring: few DMAs, all preceding every ACTIVATE; a tiny
            # ACT between them pulls the sigmoid table load forward
            for t, idx in (("xh2", 0), ("xh1", 1), ("xh2", 1), ("xh1", 3)):
                src_d, dst = srcs[t]
                nc.scalar.dma_start(out=dst[idx][:, :], in_=src_d[idx])
            # gpsimd SWDGE: ~3.5 us to first byte; later-consumed tensors
            for t, idx in (("wt1", 1), ("wt1", 2), ("wt2", 1), ("wt1", 4),
                           ("wt2", 2), ("xh1", 5), ("xh1", 6), ("wt2", 3),
                           ("wt1", 6), ("wt1", 7)):
                src_d, dst = srcs[t]
                nc.gpsimd.dma_start(out=dst[idx][:, :], in_=src_d[idx])

            # PE pre-warm: dummy matmuls on zeroed SBUF while the first input
            # DMAs are in flight, so the HAM clock-gate opens (1.2 -> 2.4 GHz)
            # before the real matmul stream begins.
            warm_sb = const_pool.tile([128, 640], bf16, tag="warm")
            nc.vector.memset(warm_sb[:, :], 0.0)
            act_probe = const_pool.tile([1, 2], f32, tag="act_probe")
            warm_ps = psum_pool.tile([128, 2048], f32, tag="pA", name="warm_ps")
            for _ in range(10):
                nc.tensor.matmul(
                    warm_ps[:, 0:512],
                    warm_sb[:, 0:128],
                    warm_sb[:, 128:640],
                    start=True,
                    stop=True,
                )
            # Hoist the sigmoid ACT_TABLE_LOAD off the serial sigmoid chain:
            # the assembler emits the table load right before this probe,
            # which runs ~8 us in, instead of right before group 0's ACT.
            nc.scalar.activation(
                act_probe[:, :],
                warm_sb[0:1, 0:2],
                mybir.ActivationFunctionType.Sigmoid,
                bias=0.0,
                scale=0.5,
            )
            # scalar ring, remainder (still before every real ACTIVATE)
            for t, idx in (("xh2", 2), ("xh2", 3)):
                src_d, dst = srcs[t]
                nc.scalar.dma_start(out=dst[idx][:, :], in_=src_d[idx])

            # Per group: eight K=128 A-matmuls back-to-back, then eight K=52
            # B-matmuls.  Identical LDWEIGHTS shapes keep the PE's
            # background-weight-buffer pull-ahead alive (~131 ns/MM vs ~390).
            store_eng = [nc.sync, nc.gpsimd, nc.gpsimd, nc.sync,
                         nc.gpsimd, nc.sync, nc.gpsimd, None]
            final_store_eng = [nc.sync, nc.gpsimd]
            for g in range(NGRP):
                b0 = (g % 2) * K2P  # B-half base partition within the pair tile
                ps = psum_pool.tile([128, 2048], f32, tag="pA", name=f"ps_{g}")
                for px in range(GP):
                    co = px * 256
                    # start=True on each 512-wide bank's first MM clears that
                    # bank's has_written bits; later MMs (start=False)
                    # overwrite fresh cells and accumulate onto written ones.
                    nc.tensor.matmul(
                        ps[:, co : co + 256],
                        wt1_sb[g][:, px * 128 : (px + 1) * 128],
                        xh1_sb[g][:, px * 256 : (px + 1) * 256],
                        start=(px % 2 == 0),
                        stop=False,
                        skip_group_check=True,
                    )
                ot = out_pool.tile([128, 2048], bf16)
                oq = oq_pool.tile([128, 2048], i8)
                for px in range(GP):
                    co = px * 256
                    nc.tensor.matmul(
                        ps[:, co : co + 256],
                        wt2_sb[g // 2][b0 : b0 + K2, px * 128 : (px + 1) * 128],
                        xh2_sb[g // 2][b0 : b0 + K2, px * 256 : (px + 1) * 256],
                        start=False,
                        stop=(px % 2 == 1),
                        skip_group_check=True,
                    )
                # Post-processing chain per group: one 4-bank sigmoid on the
                # Scalar engine (scale=0.5 descales the x*2 host pre-scale),
                # then the Vector engine quantizes to int8 for the store.
                if g == NGRP - 1:
                    # split the final group so the tail is half-sized stages
                    for h in range(2):
                        sl = slice(h * 1024, (h + 1) * 1024)
                        nc.scalar.activation(
                            ot[:, sl], ps[:, sl],
                            mybir.ActivationFunctionType.Sigmoid,
                            bias=0.0, scale=1.0 / XSCALE,
                        )
                        nc.vector.tensor_scalar(
                            oq[:, sl], ot[:, sl],
                            QSCALE, -QSCALE / 2,
                            mybir.AluOpType.mult, mybir.AluOpType.add,
                        )
                        final_store_eng[h].dma_start(
                            out=yq_d[:, g * 2048 + h * 1024 : g * 2048 + (h + 1) * 1024],
                            in_=oq[:, sl],
                        )
                else:
                    nc.scalar.activation(
                        ot[:, 0:2048], ps[:, 0:2048],
                        mybir.ActivationFunctionType.Sigmoid,
                        bias=0.0, scale=1.0 / XSCALE,
                    )
                    nc.vector.tensor_scalar(
                        oq[:, 0:2048], ot[:, 0:2048],
                        QSCALE, -QSCALE / 2,
                        mybir.AluOpType.mult, mybir.AluOpType.add,
                    )
                    store_eng[g].dma_start(
                        out=yq_d[:, g * 2048 : (g + 1) * 2048],
                        in_=oq[:, :],
                    )
                if g < 3:
                    # HAM-warmkeeping filler: dummy matmuls occupy the PE
                    # through the early input-starved gaps so the clock gate
                    # stays at 8/8.  Their garbage lands in a PSUM slot that
                    # a later group's start=True matmul clears anyway.
                    for _ in range(8):
                        nc.tensor.matmul(
                            warm_ps[:, 0:512],
                            warm_sb[:, 0:128],
                            warm_sb[:, 128:640],
                            start=True,
                            stop=True,
                        )
    nc.compile()
    return nc


TRACE = False          # set by test harness to capture an NTFF profile
LAST_RESULTS = None    # BassKernelResults of the most recent run
_NC_CACHE = None       # compiled program, reused across calls


def kernel(x, weights, nbr_idx, valid, fault_mask):
    global LAST_RESULTS
    from concourse.bass_utils import run_bass_kernel_spmd

    x = np.asarray(x)
    out_dtype = x.dtype

    W4 = _build_patch_weights(
        np.asarray(weights), np.asarray(nbr_idx), np.asarray(valid)
    ).astype(_BF16)

    # x -> zero-padded (258, 258, B) grid, fp8 e3m4 scaled by XSCALE
    xtp = np.zeros((W + 2, W + 2, BATCH), dtype=_F8E3)
    xs = np.clip(np.ascontiguousarray(x.T).astype(np.float32) * XSCALE, -15.5, 15.5)
    xtp[1 : W + 1, 1 : W + 1] = xs.astype(_F8E3).reshape(W, W, BATCH)
    # all patch hulls: (NPY, NPX, HA*HB, B)
    sl = np.lib.stride_tricks.sliding_window_view(xtp, (HA, HB), axis=(0, 1))
    hulls = (
        sl[::PA, ::PB]                      # (NPY, NPX, B, HA, HB)
        .transpose(0, 1, 3, 4, 2)
        .reshape(NPY, NPX, HA * HB, BATCH)
    )

    W4 = W4.reshape(NPY, NPX, HA * HB, 128)
    in_maps = []
    for c in range(NCORES):
        hc = hulls[c * PRPC : (c + 1) * PRPC]   # (PRPC, NPX, 180, B)
        wc = W4[c * PRPC : (c + 1) * PRPC]      # (PRPC, NPX, 180, 128)
        # half-row groups of GP=8 patches: [NGRP, 180, GP, .]
        hg = hc.reshape(NGRP, GP, HA * HB, BATCH).transpose(0, 2, 1, 3)
        wg = wc.reshape(NGRP, GP, HA * HB, 128).transpose(0, 2, 1, 3)
        # B-halves: two groups pack one 128-row tile (52 rows + 12 pad each)
        hb = np.zeros((NPAIR_G, 2, K2P, GP, BATCH), dtype=hg.dtype)
        hb[:, :, :K2] = hg[:, KSPLIT:].reshape(NPAIR_G, 2, K2, GP, BATCH)
        wb = np.zeros((NPAIR_G, 2, K2P, GP, 128), dtype=wg.dtype)
        wb[:, :, :K2] = wg[:, KSPLIT:].reshape(NPAIR_G, 2, K2, GP, 128)
        in_maps.append(
            {
                "xh1": np.ascontiguousarray(hg[:, :KSPLIT]).reshape(
                    NGRP, KSPLIT, GP * 256
                ),
                "xh2": np.ascontiguousarray(hb).reshape(NPAIR_G, 128, GP * 256),
                "wt1": np.ascontiguousarray(wg[:, :KSPLIT]).reshape(
                    NGRP, KSPLIT, GP * 128
                ),
                "wt2": np.ascontiguousarray(wb).reshape(NPAIR_G, 128, GP * 128),
            }
        )

    global _NC_CACHE
    if _NC_CACHE is None:
        _NC_CACHE = _build_program()
    nc = _NC_CACHE
    res = run_bass_kernel_spmd(
        nc, in_maps, core_ids=list(range(NCORES)), trace=TRACE
    )
    LAST_RESULTS = res

    # unshard: per-core yq is [m=oy*16+ox, NPATCH*256] int8 with patches in
    # (patch-row-major, quad) order -> dequant -> (B, HW)
    parts = []
    for c, r in enumerate(res.results):
        yq = np.asarray(r["yq"]).reshape(PA, PB, PRPC, NPX, BATCH)
        # [oy, ox, pyl, px, b] -> [b, pyl, oy, px, ox]
        parts.append(
            yq.transpose(4, 2, 0, 3, 1).reshape(BATCH, PRPC * PA, W)
        )
    yq_full = np.concatenate(parts, axis=1).reshape(BATCH, HW)
    y = (yq_full.astype(np.float32) / QSCALE + np.float32(0.5)).astype(
        out_dtype, copy=False
    )
    # faulted units: reference computes sigmoid(where(fault, y, 0)) -> 0.5
    fault = np.asarray(fault_mask).astype(bool)
    y[:, ~fault] = np.float32(0.5)
    return y


# revision 13
# speedup vs baseline: 1.4287x; 1.1336x over previous
"""Bass/Tile TRN2 kernel for a 3x3 locally-connected (unshared-weight) layer.

Computation (per batch row b, grid unit h, hw = 256*256):
    y[b,h] = sigmoid( sum_o x[b, nbr_idx[o,h]] * (valid[o,h] ? weights[o,h] : 0) )
    y[b,h] = sigmoid(0) = 0.5 where ~fault_mask[h] (mask applied pre-sigmoid)

Strategy: the gather is a fixed 3x3 stencil (verified on host at call time).
The grid (256x256) is tiled into 8x16 output patches (128 outputs = full PE
width).  A patch's 9-point stencil inputs form its 10x18 hull (180 grid
cells); with x transposed to (cell, batch), each patch is TWO matmuls:
    psum[128 out, 256 batch]  = lhsT_A[128 hull-rows, 128].T @ xh_A[128, 256]
    psum                     += lhsT_B[ 52 hull-rows, 128].T @ xh_B[ 52, 256]
where the lhsT blocks hold the (mostly zero) scattered effective weights.
The kernel is HBM-DMA-bound, so bytes-on-the-wire are the currency:
  - x hulls ship as fp8 e3m4 (x is pre-scaled by 2 on host; the ACT's
    scale=0.5 descales).  4 mantissa bits keep rel_err ~1.4e-2 < 2e-2.
  - weight blocks stay bf16 (fp8 for both operands breaks the error gate).
  - the output ships as int8: ScalarE sigmoid -> bf16, then the (otherwise
    idle) Vector engine quantizes (sigmoid*480 - 240) -> int8; the host
    dequantizes q/480 + 0.5.  Halves the 4 MiB output stream.
  - every DMA tile keeps 128 partitions: the DGE splits a transfer into
    per-partition-group descriptors and non-128 partition counts skew the
    descriptor->channel spread, hot-spotting a few of the 16 HW channels.
Traffic: ~8.0 MiB/core vs ~13.9 for the all-bf16 formulation.

Sharding: gy is split 8 ways (32 grid rows = 4 patch-rows of 16 patches per
core); batch (256) rides along the matmul free dimension.  Every core runs
an identical program; grid-boundary effects are encoded in host-built
zero-padded hulls / zero weight blocks.  All inputs are SBUF-resident and
DMA'd up-front in consumption order, balanced across the three dynamic DMA rings
(sync/scalar/vector HWDGE + gpsimd SWDGE).  Scheduling rules baked in:
matmuls run in same-shape runs so LDWEIGHTS pipelines; the Scalar/ACT
engine issues few input DMAs and no stores (a dma_start blocked on ring
capacity stalls every later ACTIVATE in its FIFO); a tiny early ACTIVATE
hoists the sigmoid ACT_TABLE_LOAD (~1.5 us) off the serial sigmoid chain;
one 4-bank ACT per group keeps that chain short; dummy matmuls after the
first three groups hold the PE's HAM clock-gate at 8/8 through the
input-starved ramp.
"""

import numpy as np
import ml_dtypes

BATCH = 256
W = 256               # grid width/height
HW = W * W
N_CONN = 9
PA, PB = 8, 16        # patch shape (gy x gx) -> M = 128 outputs
HA, HB = PA + 2, PB + 2   # hull shape 10 x 18 -> K = 180, split 128 + 52
KSPLIT = 128
K2 = HA * HB - KSPLIT     # 52
K2P = 64              # B-half padded stride: two groups pack one 128-row tile
NPAIR_G = 4           # group pairs per core
NCORES = 8
NPY, NPX = W // PA, W // PB      # 32 x 16 patch grid
PRPC = NPY // NCORES             # 4 patch-rows per core
NGRP = PRPC * 2                  # 8 half-row DMA groups (8 patches each)
GP = NPX // 2                    # patches per group
NPATCH = PRPC * NPX              # 64 patches per core

XSCALE = 2.0          # host pre-scale before e3m4 cast; ACT descales
QSCALE = 480.0        # int8 output quant: q = sigmoid*QSCALE - QSCALE/2

_BF16 = ml_dtypes.bfloat16
_F8E3 = ml_dtypes.float8_e3m4


def _build_patch_weights(weights, nbr_idx, valid):
    """Scatter effective weights into per-patch lhsT blocks.

    Returns W4 float32 (NPY*NPX, HA*HB, 128): for patch P, W4[P, k, m] is the
    weight of the connection feeding output m (= oy*16+ox) from hull cell k
    (= hy*18+hx, hull origin one cell up-left of the patch).  Raises
    ValueError if some valid (o,h) connection is not coverable.
    """
    h = np.arange(HW, dtype=np.int64)
    gy, gx = h // W, h % W
    PY, PX = gy // PA, gx // PB
    P = PY * NPX + PX
    m = (gy % PA) * PB + (gx % PB)

    g = nbr_idx.astype(np.int64)
    vm = valid.astype(bool)
    w_eff = np.where(vm, weights.astype(np.float32), 0.0)

    hy = g // W - (PA * PY - 1)
    hx = g % W - (PB * PX - 1)
    inh = (hy >= 0) & (hy < HA) & (hx >= 0) & (hx < HB)
    if not np.all(inh | ~vm):
        raise ValueError(
            "nbr_idx is not coverable by the patch-stencil kernel "
            f"({np.count_nonzero(vm & ~inh)} uncovered connections)"
        )
    k = hy * HB + hx
    mask = vm & inh
    Pb = np.broadcast_to(P, g.shape)
    mb = np.broadcast_to(m, g.shape)
    W4 = np.zeros((NPY * NPX, HA * HB, 128), dtype=np.float32)
    np.add.at(W4, (Pb[mask], k[mask], mb[mask]), w_eff[mask])
    return W4


def _build_program():
    import concourse.bacc as bacc
    import concourse.mybir as mybir
    from concourse import tile
    from concourse._compat import axon_active

    nc = bacc.Bacc(
        "TRN2",
        target_bir_lowering=False,
        debug=not axon_active(),
        num_devices=NCORES,
    )
    f32 = mybir.dt.float32
    bf16 = mybir.dt.bfloat16
    f8e3 = mybir.dt.float8e3
    i8 = mybir.dt.int8

    # Group-PAIRED input layouts: the DGE emits one descriptor per
    # partition-row per transfer and each ring sustains only ~40
    # descriptors/us, so ring bandwidth is proportional to row bytes.
    # Pairing two groups per transfer doubles row bytes (4 KB) at the
    # same descriptor count.
    xh1_d = nc.dram_tensor("xh1", [NGRP // 2, KSPLIT, 2 * GP * 256], f8e3, kind="ExternalInput")
    xh2_d = nc.dram_tensor("xh2", [NPAIR_G // 2, 128, 2 * GP * 256], f8e3, kind="ExternalInput")
    wt1_d = nc.dram_tensor("wt1", [NGRP // 2, KSPLIT, 2 * GP * 128], bf16, kind="ExternalInput")
    wt2_d = nc.dram_tensor("wt2", [NPAIR_G // 2, 128, 2 * GP * 128], bf16, kind="ExternalInput")
    yq_d = nc.dram_tensor("yq", [128, NPATCH * 256], i8, kind="ExternalOutput")

    with tile.TileContext(nc) as tc:
        with (
            tc.tile_pool(name="xh", bufs=1) as xh_pool,
            tc.tile_pool(name="wt", bufs=1) as wt_pool,
            tc.tile_pool(name="const", bufs=1) as const_pool,
            tc.tile_pool(name="out", bufs=3) as out_pool,
            tc.tile_pool(name="oq", bufs=4) as oq_pool,
            tc.tile_pool(name="psum", bufs=2, space="PSUM") as psum_pool,
        ):
            # All inputs SBUF-resident; every DMA issued up-front in
            # consumption order, balanced across the four dynamic rings.
            xh1_sb, wt1_sb, xh2_sb, wt2_sb = [], [], [], []
            for p in range(NGRP // 2):
                xh1_sb.append(xh_pool.tile([KSPLIT, 2 * GP * 256], f8e3, tag=f"xh1_{p}", name=f"xh1_{p}"))
                wt1_sb.append(wt_pool.tile([KSPLIT, 2 * GP * 128], bf16, tag=f"wt1_{p}", name=f"wt1_{p}"))
            for p in range(NPAIR_G // 2):
                xh2_sb.append(xh_pool.tile([128, 2 * GP * 256], f8e3, tag=f"xh2_{p}", name=f"xh2_{p}"))
                wt2_sb.append(wt_pool.tile([128, 2 * GP * 128], bf16, tag=f"wt2_{p}", name=f"wt2_{p}"))
            srcs = {"xh1": (xh1_d, xh1_sb), "xh2": (xh2_d, xh2_sb),
                    "wt1": (wt1_d, wt1_sb), "wt2": (wt2_d, wt2_sb)}
            # Ring split: group-0's two operands ride different rings so
            # they land in parallel; later tensors in consumption order.
            # sync ring
            for t, idx in (("xh1", 0), ("wt2", 0), ("xh1", 2), ("wt1", 2)):
                src_d, dst = srcs[t]
                nc.sync.dma_start(out=dst[idx][:, :], in_=src_d[idx])
            # scalar ring: input DMAs only ever <=4 outstanding, so they
            # never capacity-stall the ACTIVATE chain behind them
            for t, idx in (("xh2", 0), ("wt1", 1), ("xh2", 1), ("wt2", 1),
                           ("wt1", 3)):
                src_d, dst = srcs[t]
                nc.scalar.dma_start(out=dst[idx][:, :], in_=src_d[idx])
            # gpsimd SWDGE: ~3.5 us to first byte, ~31 descriptors/us
            for t, idx in (("wt1", 0), ("xh1", 1), ("xh1", 3)):
                src_d, dst = srcs[t]
                nc.gpsimd.dma_start(out=dst[idx][:, :], in_=src_d[idx])

            # PE pre-warm: dummy matmuls on zeroed SBUF while the first input
            # DMAs are in flight, so the HAM clock-gate opens (1.2 -> 2.4 GHz)
            # before the real matmul stream begins.
            warm_sb = const_pool.tile([128, 640], bf16, tag="warm")
            nc.vector.memset(warm_sb[:, :], 0.0)
            act_probe = const_pool.tile([1, 2], f32, tag="act_probe")
            warm_ps = psum_pool.tile([128, 2048], f32, tag="pA", name="warm_ps")
            for _ in range(5):
                nc.tensor.matmul(
                    warm_ps[:, 0:512],
                    warm_sb[:, 0:128],
                    warm_sb[:, 128:640],
                    start=True,
                    stop=True,
                )
            # Hoist the sigmoid ACT_TABLE_LOAD off the serial sigmoid chain:
            # the assembler emits the table load right before this probe,
            # which runs ~8 us in, instead of right before group 0's ACT.
            nc.scalar.activation(
                act_probe[:, :],
                warm_sb[0:1, 0:2],
                mybir.ActivationFunctionType.Sigmoid,
                bias=0.0,
                scale=0.5,
            )
            # Per group: eight K=128 A-matmuls back-to-back, then eight K=52
            # B-matmuls.  Identical LDWEIGHTS shapes keep the PE's
            # background-weight-buffer pull-ahead alive (~131 ns/MM vs ~390).
            # Stores pair two groups into one 4 KB-row transfer (g6 and the
            # split g7 stay separate so the tail is half-sized stages).
            pair_store_eng = [nc.sync, nc.gpsimd, nc.gpsimd]
            final_store_eng = [nc.sync, nc.gpsimd]
            oq = None
            for g in range(NGRP):
                b0 = (g % 2) * K2P  # B-half base partition within the pair tile
                c1 = (g % 2) * 2048       # column base within paired A tiles
                c1w = (g % 2) * 1024
                c2 = ((g // 2) % 2) * 2048  # column base within merged B tiles
                c2w = ((g // 2) % 2) * 1024
                ps = psum_pool.tile([128, 2048], f32, tag="pA", name=f"ps_{g}")
                for px in range(GP):
                    co = px * 256
                    # start=True on each 512-wide bank's first MM clears that
                    # bank's has_written bits; later MMs (start=False)
                    # overwrite fresh cells and accumulate onto written ones.
                    nc.tensor.matmul(
                        ps[:, co : co + 256],
                        wt1_sb[g // 2][:, c1w + px * 128 : c1w + (px + 1) * 128],
                        xh1_sb[g // 2][:, c1 + px * 256 : c1 + (px + 1) * 256],
                        start=(px % 2 == 0),
                        stop=False,
                        skip_group_check=True,
                    )
                ot = out_pool.tile([128, 2048], bf16)
                if g % 2 == 0:
                    oq = oq_pool.tile([128, 4096], i8)
                qcol = (g % 2) * 2048
                for px in range(GP):
                    co = px * 256
                    nc.tensor.matmul(
                        ps[:, co : co + 256],
                        wt2_sb[g // 4][b0 : b0 + K2, c2w + px * 128 : c2w + (px + 1) * 128],
                        xh2_sb[g // 4][b0 : b0 + K2, c2 + px * 256 : c2 + (px + 1) * 256],
                        start=False,
                        stop=(px % 2 == 1),
                        skip_group_check=True,
                    )
                # Post-processing chain per group: one 4-bank sigmoid on the
                # Scalar engine (scale=0.5 descales the x*2 host pre-scale),
                # then the Vector engine quantizes to int8 for the store.
                if g == NGRP - 1:
                    # split the final group so the tail is half-sized stages
                    for h in range(2):
                        sl = slice(h * 1024, (h + 1) * 1024)
                        qsl = slice(qcol + h * 1024, qcol + (h + 1) * 1024)
                        nc.scalar.activation(
                            ot[:, sl], ps[:, sl],
                            mybir.ActivationFunctionType.Sigmoid,
                            bias=0.0, scale=1.0 / XSCALE,
                        )
                        nc.vector.tensor_scalar(
                            oq[:, qsl], ot[:, sl],
                            QSCALE, -QSCALE / 2,
                            mybir.AluOpType.mult, mybir.AluOpType.add,
                        )
                        final_store_eng[h].dma_start(
                            out=yq_d[:, g * 2048 + h * 1024 : g * 2048 + (h + 1) * 1024],
                            in_=oq[:, qsl],
                        )
                else:
                    nc.scalar.activation(
                        ot[:, 0:2048], ps[:, 0:2048],
                        mybir.ActivationFunctionType.Sigmoid,
                        bias=0.0, scale=1.0 / XSCALE,
                    )
                    nc.vector.tensor_scalar(
                        oq[:, qcol : qcol + 2048], ot[:, 0:2048],
                        QSCALE, -QSCALE / 2,
                        mybir.AluOpType.mult, mybir.AluOpType.add,
                    )
                    if g == NGRP - 2:
                        # g6 stores alone (g7 is the split tail)
                        nc.sync.dma_start(
                            out=yq_d[:, g * 2048 : (g + 1) * 2048],
                            in_=oq[:, qcol : qcol + 2048],
                        )
                    elif g % 2 == 1:
                        pair_store_eng[g // 2].dma_start(
                            out=yq_d[:, (g - 1) * 2048 : (g + 1) * 2048],
                            in_=oq[:, :],
                        )
                if g < 3:
                    # HAM-warmkeeping filler: dummy matmuls occupy the PE
                    # through the early input-starved gaps so the clock gate
                    # stays at 8/8.  Their garbage lands in a PSUM slot that
                    # a later group's start=True matmul clears anyway.
                    for _ in range(4):
                        nc.tensor.matmul(
                            warm_ps[:, 0:512],
                            warm_sb[:, 0:128],
                            warm_sb[:, 128:640],
                            start=True,
                            stop=True,
                        )
    nc.compile()
    return nc


TRACE = False          # set by test harness to capture an NTFF profile
LAST_RESULTS = None    # BassKernelResults of the most recent run
_NC_CACHE = None       # compiled program, reused across calls


def kernel(x, weights, nbr_idx, valid, fault_mask):
    global LAST_RESULTS
    from concourse.bass_utils import run_bass_kernel_spmd

    x = np.asarray(x)
    out_dtype = x.dtype

    W4 = _build_patch_weights(
        np.asarray(weights), np.asarray(nbr_idx), np.asarray(valid)
    ).astype(_BF16)

    # x -> zero-padded (258, 258, B) grid, fp8 e3m4 scaled by XSCALE
    xtp = np.zeros((W + 2, W + 2, BATCH), dtype=_F8E3)
    xs = np.clip(np.ascontiguousarray(x.T).astype(np.float32) * XSCALE, -15.5, 15.5)
    xtp[1 : W + 1, 1 : W + 1] = xs.astype(_F8E3).reshape(W, W, BATCH)
    # all patch hulls: (NPY, NPX, HA*HB, B)
    sl = np.lib.stride_tricks.sliding_window_view(xtp, (HA, HB), axis=(0, 1))
    hulls = (
        sl[::PA, ::PB]                      # (NPY, NPX, B, HA, HB)
        .transpose(0, 1, 3, 4, 2)
        .reshape(NPY, NPX, HA * HB, BATCH)
    )

    W4 = W4.reshape(NPY, NPX, HA * HB, 128)
    in_maps = []
    for c in range(NCORES):
        hc = hulls[c * PRPC : (c + 1) * PRPC]   # (PRPC, NPX, 180, B)
        wc = W4[c * PRPC : (c + 1) * PRPC]      # (PRPC, NPX, 180, 128)
        # half-row groups of GP=8 patches: [NGRP, 180, GP, .]
        hg = hc.reshape(NGRP, GP, HA * HB, BATCH).transpose(0, 2, 1, 3)
        wg = wc.reshape(NGRP, GP, HA * HB, 128).transpose(0, 2, 1, 3)
        # B-halves: two groups pack one 128-row tile (52 rows + 12 pad each)
        hb = np.zeros((NPAIR_G, 2, K2P, GP, BATCH), dtype=hg.dtype)
        hb[:, :, :K2] = hg[:, KSPLIT:].reshape(NPAIR_G, 2, K2, GP, BATCH)
        wb = np.zeros((NPAIR_G, 2, K2P, GP, 128), dtype=wg.dtype)
        wb[:, :, :K2] = wg[:, KSPLIT:].reshape(NPAIR_G, 2, K2, GP, 128)
        xh1g = np.ascontiguousarray(hg[:, :KSPLIT]).reshape(NGRP, KSPLIT, GP * 256)
        wt1g = np.ascontiguousarray(wg[:, :KSPLIT]).reshape(NGRP, KSPLIT, GP * 128)
        xh2g = np.ascontiguousarray(hb).reshape(NPAIR_G, 128, GP * 256)
        wt2g = np.ascontiguousarray(wb).reshape(NPAIR_G, 128, GP * 128)
        # pair adjacent groups along the free dim: 4 KB descriptor rows
        in_maps.append(
            {
                "xh1": np.ascontiguousarray(
                    np.concatenate([xh1g[0::2], xh1g[1::2]], axis=2)
                ),
                "xh2": np.ascontiguousarray(
                    np.concatenate([xh2g[0::2], xh2g[1::2]], axis=2)
                ),
                "wt1": np.ascontiguousarray(
                    np.concatenate([wt1g[0::2], wt1g[1::2]], axis=2)
                ),
                "wt2": np.ascontiguousarray(
                    np.concatenate([wt2g[0::2], wt2g[1::2]], axis=2)
                ),
            }
        )

    global _NC_CACHE
    if _NC_CACHE is None:
        _NC_CACHE = _build_program()
    nc = _NC_CACHE
    res = run_bass_kernel_spmd(
        nc, in_maps, core_ids=list(range(NCORES)), trace=TRACE
    )
    LAST_RESULTS = res

    # unshard: per-core yq is [m=oy*16+ox, NPATCH*256] int8 with patches in
    # (patch-row-major, quad) order -> dequant -> (B, HW)
    parts = []
    for c, r in enumerate(res.results):
        yq = np.asarray(r["yq"]).reshape(PA, PB, PRPC, NPX, BATCH)
        # [oy, ox, pyl, px, b] -> [b, pyl, oy, px, ox]
        parts.append(
            yq.transpose(4, 2, 0, 3, 1).reshape(BATCH, PRPC * PA, W)
        )
    yq_full = np.concatenate(parts, axis=1).reshape(BATCH, HW)
    y = (yq_full.astype(np.float32) / QSCALE + np.float32(0.5)).astype(
        out_dtype, copy=False
    )
    # faulted units: reference computes sigmoid(where(fault, y, 0)) -> 0.5
    fault = np.asarray(fault_mask).astype(bool)
    y[:, ~fault] = np.float32(0.5)
    return y
